# revision 1
# baseline (speedup 1.0000x reference)
import sys

sys.path.insert(0, "/opt/trn_rl_repo")

import numpy as np

import concourse.bass as bass
import concourse.tile as tile
from concourse import mybir
from concourse.bass_utils import run_bass_kernel_spmd

B, N, H, DK = 1024, 36, 16, 64
D = H * DK
NCORES = 8
BPC = B // NCORES          # batches per core
TPC = BPC * N              # tokens per core = 4608
THALF = TPC // 2           # 2304 tokens per call
TT = 128                   # token tile
NTT = THALF // TT          # 18 token tiles per call

_NC_CACHE = {}


def _build_proj_nc():
    """y[t, e] = x.T @ w for x [D, THALF] feature-major (host-packed
    [128, 8*THALF]), w [D, D] (host-packed [128, 8*D]). Single-shot: every
    SBUF tile written once so each DMA carries at most one sync wait
    (walrus here runs with DynamicDMA disabled -> 1-wait DIRECT2D structs)."""
    if "nc" in _NC_CACHE:
        return _NC_CACHE["nc"]
    FW = THALF + D
    NG = NTT * 2  # 36 psum groups
    nc = bass.Bass()
    xw = nc.dram_tensor("xw", [128, 8 * FW], mybir.dt.float32r,
                        kind="ExternalInput")
    y = nc.dram_tensor("y", [128, NTT * D], mybir.dt.float32,
                       kind="ExternalOutput")

    with (
        nc.sbuf_tensor("xw_sb", [128, 8, FW], mybir.dt.float32r) as xw_sb,
        nc.sbuf_tensor("y_sb", [128, NTT, D], mybir.dt.float32) as y_sb,
        nc.psum_tensor("ps", [128, 4, 512], mybir.dt.float32) as ps,
        nc.semaphore("dma_sem") as dma_sem,
        nc.semaphore("pe_sem") as pe_sem,
        nc.semaphore("act_sem") as act_sem,
        nc.Block() as block,
    ):
        @block.gpsimd
        def _(g):
            g.dma_start(
                out=xw_sb[:],
                in_=xw.rearrange("p (c t) -> p c t", c=8),
            ).then_inc(dma_sem, 16)
            g.wait_ge(act_sem, NG)
            g.dma_start(out=y[:, :], in_=y_sb[:]).then_inc(dma_sem, 16)

        @block.tensor
        def _(te):
            te.wait_ge(dma_sem, 16)
            for j in range(NG):
                t, eh = j // 2, j % 2
                if j >= 4:
                    te.wait_ge(act_sem, j - 3)
                for c in range(8):
                    mm = te.matmul(
                        ps[:, j % 4, :],
                        lhsT=xw_sb[:, c, t * TT:(t + 1) * TT],
                        rhs=xw_sb[:, c,
                                  THALF + eh * 512:THALF + (eh + 1) * 512],
                        start=(c == 0),
                        stop=(c == 7),
                    )
                    if c == 7:
                        mm.then_inc(pe_sem, 1)

        @block.scalar
        def _(sc):
            for j in range(NG):
                t, eh = j // 2, j % 2
                sc.wait_ge(pe_sem, j + 1)
                sc.copy(
                    out=y_sb[:, t, eh * 512:(eh + 1) * 512],
                    in_=ps[:, j % 4, :],
                ).then_inc(act_sem, 1)
    _NC_CACHE["nc"] = nc
    return nc


def _pack_xw(xs_td: np.ndarray, w_pdD: np.ndarray) -> np.ndarray:
    """x [THALF, D] token-major + pre-chunked w [128, 8, D]
    -> [128, 8*(THALF+D)] with per-chunk [xT_slice | w_slice]."""
    xT = xs_td.T.reshape(8, 128, THALF).transpose(1, 0, 2)  # [p, c, t]
    out = np.empty((128, 8, THALF + D), np.float32)
    out[:, :, :THALF] = xT
    out[:, :, THALF:] = w_pdD
    return out.reshape(128, 8 * (THALF + D))


def _pack_w(w: np.ndarray) -> np.ndarray:
    return np.ascontiguousarray(w.reshape(8, 128, D).transpose(1, 0, 2))


def _unpack_y(y2: np.ndarray) -> np.ndarray:
    """[128, NTT*D] -> [THALF, D]"""
    return np.ascontiguousarray(
        y2.reshape(128, NTT, D).transpose(1, 0, 2).reshape(THALF, D))


def _proj_spmd(x_bnd: np.ndarray, w: np.ndarray) -> np.ndarray:
    """x [B, N, D] @ w [D, D] on 8 cores, batch-sharded. Returns [B, N, D]."""
    nc = _build_proj_nc()
    wp = _pack_w(w)
    out = np.empty((B * N, D), np.float32)
    for half in range(2):
        in_maps = []
        for c in range(NCORES):
            xs = x_bnd[c * BPC:(c + 1) * BPC].reshape(TPC, D)
            in_maps.append({
                "xw": _pack_xw(xs[half * THALF:(half + 1) * THALF], wp),
            })
        res = run_bass_kernel_spmd(nc, in_maps, core_ids=list(range(NCORES)))
        for c in range(NCORES):
            t0 = c * TPC + half * THALF
            out[t0:t0 + THALF] = _unpack_y(res.results[c]["y"])
    return out.reshape(B, N, D)


def _box_relational_embedding(f_g):
    x_min, y_min, x_max, y_max = np.split(f_g.astype(np.float32), 4, axis=-1)
    cx = (x_min + x_max) * 0.5
    cy = (y_min + y_max) * 0.5
    w = x_max - x_min + 1.0
    h = y_max - y_min + 1.0
    dx = np.log(np.clip(np.abs((cx - np.swapaxes(cx, 1, 2)) / w), 1e-3, None))
    dy = np.log(np.clip(np.abs((cy - np.swapaxes(cy, 1, 2)) / h), 1e-3, None))
    dw = np.log(w / np.swapaxes(w, 1, 2))
    dh = np.log(h / np.swapaxes(h, 1, 2))
    pos = np.stack([dx, dy, dw, dh], axis=-1)  # [B, N, N, 4]
    dim_mat = 1.0 / (1000.0 ** (np.arange(8, dtype=np.float32) / 8.0))
    mul = (100.0 * pos)[..., None] * dim_mat
    mul = mul.reshape(pos.shape[0], N, N, 32).astype(np.float32)
    return np.concatenate([np.sin(mul), np.cos(mul)], axis=-1)


def kernel(input_query, input_key, input_value, input_box,
           Wq, bq, Wk, bk, Wv, bv, Wo, bo, Wg, bg, Wa, ba):
    f32 = np.float32
    q = np.asarray(input_query, f32)
    k = np.asarray(input_key, f32)
    v = np.asarray(input_value, f32)
    box = np.asarray(input_box, f32)

    # device: the three input projections (batch-sharded over 8 cores)
    qh = _proj_spmd(q, np.asarray(Wq, f32)) + np.asarray(bq, f32)
    kh = _proj_spmd(k, np.asarray(Wk, f32)) + np.asarray(bk, f32)
    vh = _proj_spmd(v, np.asarray(Wv, f32)) + np.asarray(bv, f32)
    qh = qh.reshape(B, N, H, DK).transpose(0, 2, 1, 3)  # [B,H,N,DK]
    kh = kh.reshape(B, N, H, DK).transpose(0, 2, 1, 3)
    vh = vh.reshape(B, N, H, DK).transpose(0, 2, 1, 3)

    Wg_ = np.asarray(Wg, f32)
    bg_ = np.asarray(bg, f32)
    Wa_ = np.asarray(Wa, f32)
    ba_ = np.asarray(ba, f32)

    out_pre = np.empty((B, N, D), f32)
    CH = 128
    for b0 in range(0, B, CH):
        b1 = b0 + CH
        emb = _box_relational_embedding(box[b0:b1])      # [CH, N, N, 64]
        rel = np.einsum("bnmg,hg->bhnm", emb, Wg_) + bg_[None, :, None, None]
        rel = np.maximum(rel, 0.0)
        qc, kc, vc = qh[b0:b1], kh[b0:b1], vh[b0:b1]
        alpha = qc @ Wa_ + ba_                            # [CH,H,N,N]
        w_g = (alpha * rel).sum(axis=2)[:, :, None, :]    # [CH,H,1,N]
        scores = np.einsum("bhnd,bhmd->bhnm", qc, kc) / np.sqrt(DK)
        logits = np.log(np.clip(w_g, 1e-6, None)) + scores
        logits -= logits.max(axis=-1, keepdims=True)
        e = np.exp(logits)
        wmn = e / e.sum(axis=-1, keepdims=True)
        o = np.einsum("bhnm,bhmd->bhnd", wmn, vc)         # [CH,H,N,DK]
        out_pre[b0:b1] = o.transpose(0, 2, 1, 3).reshape(CH, N, D)

    # device: output projection
    out = _proj_spmd(out_pre, np.asarray(Wo, f32)) + np.asarray(bo, f32)
    return out.astype(f32)



# revision 2
# speedup vs baseline: 7.6213x; 7.6213x over previous
import sys

sys.path.insert(0, "/opt/trn_rl_repo")

import ml_dtypes
import numpy as np

import concourse.bass as bass
from concourse import mybir
from concourse.bass_utils import run_bass_kernel_spmd

B, N, H, DK = 1024, 36, 16, 64
D = H * DK
NCORES = 8
BPC = B // NCORES          # batches per core
TPC = BPC * N              # tokens per core = 4608
THALF = TPC                # full-token launches (kept for test.py compat)
TT = 128                   # token tile
NTT = TPC // TT            # 36 token tiles per launch
BF16 = ml_dtypes.bfloat16

_NC_CACHE = {}


def _build_proj_nc():
    """y[t, e] = x.T @ w in bf16 (fp32 PSUM accum) for the full 4608-token
    per-core shard in one launch. Host packs xT [128, 8, TPC] and w
    [128, 8, D] into one bf16 input so the kernel is a single DMA in /
    matmul sweep / single DMA out. Same explicit-semaphore structure as the
    f32r half-token baseline, but half the tunnel bytes and half the
    launches."""
    if "nc" in _NC_CACHE:
        return _NC_CACHE["nc"]
    FW = TPC + D
    NG = NTT * 2  # 72 psum groups (token tile x output half)
    nc = bass.Bass()
    xw = nc.dram_tensor("xw", [128, 8 * FW], mybir.dt.bfloat16,
                        kind="ExternalInput")
    y = nc.dram_tensor("y", [128, NTT * D], mybir.dt.bfloat16,
                       kind="ExternalOutput")

    with (
        nc.sbuf_tensor("xw_sb", [128, 8, FW], mybir.dt.bfloat16) as xw_sb,
        nc.sbuf_tensor("y_sb", [128, NTT, D], mybir.dt.bfloat16) as y_sb,
        nc.psum_tensor("ps", [128, 4, 512], mybir.dt.float32) as ps,
        nc.semaphore("dma_sem") as dma_sem,
        nc.semaphore("pe_sem") as pe_sem,
        nc.semaphore("act_sem") as act_sem,
        nc.Block() as block,
    ):
        @block.gpsimd
        def _(g):
            g.dma_start(
                out=xw_sb[:],
                in_=xw.rearrange("p (c t) -> p c t", c=8),
            ).then_inc(dma_sem, 16)
            g.wait_ge(act_sem, NG)
            g.dma_start(out=y[:, :], in_=y_sb[:]).then_inc(dma_sem, 16)

        @block.tensor
        def _(te):
            te.wait_ge(dma_sem, 16)
            for j in range(NG):
                t, eh = j // 2, j % 2
                if j >= 4:
                    te.wait_ge(act_sem, j - 3)
                for c in range(8):
                    mm = te.matmul(
                        ps[:, j % 4, :],
                        lhsT=xw_sb[:, c, t * TT:(t + 1) * TT],
                        rhs=xw_sb[:, c,
                                  TPC + eh * 512:TPC + (eh + 1) * 512],
                        start=(c == 0),
                        stop=(c == 7),
                    )
                    if c == 7:
                        mm.then_inc(pe_sem, 1)

        @block.scalar
        def _(sc):
            for j in range(NG):
                t, eh = j // 2, j % 2
                sc.wait_ge(pe_sem, j + 1)
                sc.copy(
                    out=y_sb[:, t, eh * 512:(eh + 1) * 512],
                    in_=ps[:, j % 4, :],
                ).then_inc(act_sem, 1)
    _NC_CACHE["nc"] = nc
    return nc


def _pack_xw(xs_td: np.ndarray, w_pdD: np.ndarray) -> np.ndarray:
    """x [TPC, D] token-major + pre-chunked bf16 w [128, 8, D]
    -> bf16 [128, 8*(TPC+D)] with per-chunk [xT_slice | w_slice]."""
    xT = np.ascontiguousarray(xs_td.T).astype(BF16)
    xT = xT.reshape(8, 128, TPC).transpose(1, 0, 2)  # [p, c, t]
    out = np.empty((128, 8, TPC + D), BF16)
    out[:, :, :TPC] = xT
    out[:, :, TPC:] = w_pdD
    return out.reshape(128, 8 * (TPC + D))


def _pack_w(w: np.ndarray) -> np.ndarray:
    return np.ascontiguousarray(
        w.reshape(8, 128, D).transpose(1, 0, 2)).astype(BF16)


def _unpack_y(y2: np.ndarray) -> np.ndarray:
    """bf16 [128, NTT*D] -> f32 [TPC, D]"""
    return np.ascontiguousarray(
        y2.reshape(128, NTT, D).transpose(1, 0, 2)).astype(
            np.float32).reshape(TPC, D)


def _proj_spmd(x_bnd: np.ndarray, w: np.ndarray) -> np.ndarray:
    """x [B, N, D] @ w [D, D] on 8 cores, batch-sharded, one launch.
    Returns [B, N, D] f32."""
    nc = _build_proj_nc()
    wp = _pack_w(w)
    in_maps = []
    for c in range(NCORES):
        xs = x_bnd[c * BPC:(c + 1) * BPC].reshape(TPC, D)
        in_maps.append({"xw": _pack_xw(xs, wp)})
    res = run_bass_kernel_spmd(nc, in_maps, core_ids=list(range(NCORES)))
    out = np.empty((B * N, D), np.float32)
    for c in range(NCORES):
        out[c * TPC:(c + 1) * TPC] = _unpack_y(res.results[c]["y"])
    return out.reshape(B, N, D)


def kernel(input_query, input_key, input_value, input_box,
           Wq, bq, Wk, bk, Wv, bv, Wo, bo, Wg, bg, Wa, ba):
    f32 = np.float32
    q = np.asarray(input_query, f32)
    k = np.asarray(input_key, f32)
    v = np.asarray(input_value, f32)
    box = np.asarray(input_box, f32)

    # device: the three input projections (batch-sharded over 8 cores)
    qh = _proj_spmd(q, np.asarray(Wq, f32)) + np.asarray(bq, f32)
    kh = _proj_spmd(k, np.asarray(Wk, f32)) + np.asarray(bk, f32)
    vh = _proj_spmd(v, np.asarray(Wv, f32)) + np.asarray(bv, f32)
    qh = qh.reshape(B, N, H, DK).transpose(0, 2, 1, 3)  # [B,H,N,DK]
    kh = kh.reshape(B, N, H, DK).transpose(0, 2, 1, 3)
    vh = vh.reshape(B, N, H, DK).transpose(0, 2, 1, 3)

    Wg_ = np.asarray(Wg, f32)
    bg_ = np.asarray(bg, f32)
    Wa_ = np.asarray(Wa, f32)
    ba_ = np.asarray(ba, f32)
    Wg_s, Wg_c = Wg_[:, :32], Wg_[:, 32:]
    dim_mat = (1.0 / (1000.0 ** (np.arange(8, dtype=f32) / 8.0))).astype(f32)

    # host attention: everything phrased as batched BLAS matmuls
    out_pre = np.empty((B, N, D), f32)
    CH = 128
    inv_sqrt_dk = f32(1.0 / np.sqrt(DK))
    for b0 in range(0, B, CH):
        b1 = b0 + CH
        bx = box[b0:b1]
        x_min, y_min = bx[..., 0:1], bx[..., 1:2]
        x_max, y_max = bx[..., 2:3], bx[..., 3:4]
        cx = (x_min + x_max) * 0.5
        cy = (y_min + y_max) * 0.5
        w = x_max - x_min + 1.0
        h = y_max - y_min + 1.0
        dcx = cx - cx.transpose(0, 2, 1)
        dcy = cy - cy.transpose(0, 2, 1)
        dx = np.log(np.clip(np.abs(dcx) / w, 1e-3, None))
        dy = np.log(np.clip(np.abs(dcy) / h, 1e-3, None))
        lw = np.log(w)
        lh = np.log(h)
        dw = lw - lw.transpose(0, 2, 1)
        dh = lh - lh.transpose(0, 2, 1)
        pos = np.stack([dx, dy, dw, dh], axis=-1)             # [CH,n,m,4]
        mul = ((100.0 * pos)[..., None] * dim_mat).reshape(-1, 32)
        rel_flat = np.sin(mul) @ Wg_s.T + np.cos(mul) @ Wg_c.T
        rel = rel_flat.reshape(CH, N, N, H).transpose(0, 3, 1, 2)
        rel = np.maximum(rel + bg_[None, :, None, None], 0.0)  # [CH,H,n,m]

        qc, kc, vc = qh[b0:b1], kh[b0:b1], vh[b0:b1]
        alpha = qc @ Wa_ + ba_                                 # [CH,H,n,m]
        w_g = np.einsum('bhnm,bhnm->bhm', alpha, rel)
        scores = (qc @ kc.transpose(0, 1, 3, 2)) * inv_sqrt_dk
        logits = np.log(np.clip(w_g, 1e-6, None))[:, :, None, :] + scores
        logits -= logits.max(-1, keepdims=True)
        e = np.exp(logits)
        wmn = e / e.sum(-1, keepdims=True)
        o = wmn @ vc                                           # [CH,H,n,DK]
        out_pre[b0:b1] = o.transpose(0, 2, 1, 3).reshape(CH, N, D)

    # device: output projection
    out = _proj_spmd(out_pre, np.asarray(Wo, f32)) + np.asarray(bo, f32)
    return out.astype(f32)
